# revision 37
# baseline (speedup 1.0000x reference)
"""DressedQuantumNet on 8 TRN2 NeuronCores (pure data parallel).

Math: pre-net angles th = X @ pre_w.T + pre_b.  After the H+RY(th) layer the
state is the real product state psi = kron_w u_w with
u_w = (cos(th_w/2) - sin(th_w/2), cos(th_w/2) + sin(th_w/2)) / sqrt(2),
and the rest of the circuit is a FIXED unitary V (depends only on q_weights).
Hence out_k = psi^T C_k psi + post_b_k with C_k = sum_w post_w[k,w]
Re(V^H Z_w V) real-symmetric.  Eigendecompose C_k = Q_k L_k Q_k^T on host:

  out_k = sum_r lam'_{k,r} * (Q_k^T psi')_r^2,   lam' = (lam + post_b_k)/16

using the unnormalized psi' (norm^2 = 16 exactly since each |u'_w|^2 = 2).

Device pipeline per 1024-row slab (batch rows on SBUF partitions):
  DMA fp16 X^T chunks -> PE matmul (X stationary, [W/2 | W/2] moving) ->
  angles in PSUM -> DVE bias-add + range-wrap -> ScalarE Sin LUT ->
  DVE psi' build (5 small fp16 ops) -> DMA-xbar transpose [128,128] ->
  PE matmul with block-diag [Q_0|Q_1] (4 tiles per matmul) -> ScalarE square
  -> PE matmul with block-diag lambda -> ScalarE copy -> DMA out.

This keeps the 81-term polynomial contraction OFF the (slow) vector engine:
the only DVE work is ~9 small elementwise ops per slab.  fp16 X halves HBM
traffic vs fp32 (theta error ~1e-3 << 2e-2 gate).
"""

from contextlib import ExitStack

import numpy as np

import concourse.bass as bass
import concourse.bacc as bacc_mod
import concourse.mybir as mybir
from concourse.bass_utils import run_bass_kernel_spmd
from concourse.tile import TileContext

N_CORES = 8
B_TOTAL = 65536
F_IN = 512
ROWS = B_TOTAL // N_CORES   # 8192 rows per core
P = 128
N_TILES = ROWS // P         # 64 row-tiles
SLAB = 8
N_SLABS = N_TILES // SLAB   # 8
N_PAIRS = N_SLABS // 2      # 4 slab-pairs (2048 rows each)
HTILES = 4                  # tiles per input DMA chunk (2 chunks per slab)

F32 = mybir.dt.float32
FP16 = mybir.dt.float16
PI = float(np.pi)

N_QUBITS, VAR_DEPTH = 4, 3


# ----------------------------------------------------------------- host math
def _gate_1q(g, w):
    ops = [np.eye(2, dtype=complex)] * N_QUBITS
    ops[w] = g
    U = ops[0]
    for i in range(1, N_QUBITS):
        U = np.kron(U, ops[i])
    return U


def _bit(i, w):  # wire 0 = most significant
    return (i >> (N_QUBITS - 1 - w)) & 1


def _cnot(c, t):
    M = np.zeros((16, 16), dtype=complex)
    for i in range(16):
        j = i ^ (1 << (N_QUBITS - 1 - t)) if _bit(i, c) else i
        M[j, i] = 1.0
    return M


def _ry(theta):
    c, s = np.cos(theta / 2), np.sin(theta / 2)
    return np.array([[c, -s], [s, c]], dtype=complex)


def _rz(theta):
    ph = np.exp(1j * theta / 2)
    return np.array([[np.conj(ph), 0], [0, ph]], dtype=complex)


def _fixed_unitary(qw):
    V = np.eye(16, dtype=complex)

    def app(Gm):
        nonlocal V
        V = Gm @ V

    def entangle():
        app(_cnot(0, 1)); app(_cnot(2, 3)); app(_cnot(1, 2))

    for k in range(VAR_DEPTH):
        entangle()
        for w in range(N_QUBITS):
            app(_gate_1q(_ry(qw[k, w]), w))
        for w in range(N_QUBITS):
            app(_gate_1q(_rz(qw[k, w]), w))
    for k in range(VAR_DEPTH):
        entangle()
        for w in range(N_QUBITS):
            app(_gate_1q(_ry(qw[k, w]), w))
        for w in range(N_QUBITS):
            app(_gate_1q(_rz(qw[3 + k, w]), w))
    entangle()
    return V


def _eigen_consts(q_weights, post_w, post_b):
    """G [16,32] = [Q_0 | Q_1]; lamcol [32,2]: (lam_k + post_b_k)/16."""
    V = _fixed_unitary(np.asarray(q_weights, dtype=np.float64))
    Gcols = []
    lamcol = np.zeros((32, 2), np.float64)
    for k in range(2):
        C = np.zeros((16, 16), dtype=complex)
        for w in range(N_QUBITS):
            z = np.array([1.0 - 2.0 * _bit(i, w) for i in range(16)])
            C += post_w[k, w] * (V.conj().T @ np.diag(z) @ V)
        M = C.real
        M = (M + M.T) / 2
        lam, Q = np.linalg.eigh(M)
        Gcols.append(Q)
        lamcol[16 * k:16 * (k + 1), k] = (lam + post_b[k]) / 16.0
    G = np.concatenate(Gcols, axis=1)  # [16, 32]
    return G, lamcol


# ------------------------------------------------------------- device kernel
def build_bass():
    nc = bacc_mod.Bacc(None, target_bir_lowering=False)
    # inputs: packed transposed fp16 X; per chunk c: [P, 4, 512] with
    # pack[p,k,b] = X16[c*512 + b, 128k + p]
    x_d = nc.dram_tensor("xtp", [ROWS * F_IN], FP16, kind="ExternalInput")
    # fp16 blob: [wpk(32)|gba(128)|gbb(128)|lam(8)|ident(128)|sel(32)] = [P,456]
    ch_d = nc.dram_tensor("cblob", [P, 456], FP16, kind="ExternalInput")
    b2_d = nc.dram_tensor("bias2", [P, 2, 4], F32, kind="ExternalInput")
    # out[2j+k, pp, (2u+v)*128+p] = out_row(pp*2048 + j*512 + (2u+v)*128 + p, k)
    o_d = nc.dram_tensor("out", [8, N_PAIRS, 512], F32, kind="ExternalOutput")

    CHUNK_ELEMS = P * 4 * HTILES * P  # 262144 elems per DMA chunk

    with TileContext(nc) as tc, ExitStack() as ctx:
        # constants arrive as one fp16 blob + one f32 blob (2 DMA issues)
        const = ctx.enter_context(tc.tile_pool(name="const", bufs=1))
        ch = const.tile([P, 456], FP16)
        nc.scalar.dma_start(ch, ch_d[:])
        wpk = ch[:, 0:32].rearrange("p (k j) -> p k j", k=4)
        gba = ch[:, 32:160]
        gbb = ch[:, 160:288]
        lam = ch[:, 288:296]
        ident = ch[:, 296:424]
        sel = ch[:, 424:456]
        bia = const.tile([P, 2, 4], F32)
        nc.scalar.dma_start(bia, b2_d[:])

        xp = ctx.enter_context(tc.tile_pool(name="xin", bufs=16))
        ttp = ctx.enter_context(tc.tile_pool(name="ttp", bufs=1, space="PSUM"))
        tsb = ctx.enter_context(tc.tile_pool(name="tsb", bufs=3))
        angp = ctx.enter_context(tc.tile_pool(name="angp", bufs=1, space="PSUM"))
        scr = ctx.enter_context(tc.tile_pool(name="scr", bufs=2))
        qp = ctx.enter_context(tc.tile_pool(name="qp", bufs=3))
        ptm = ctx.enter_context(tc.tile_pool(name="ptm", bufs=2, space="PSUM"))
        ptp = ctx.enter_context(tc.tile_pool(name="ptp", bufs=2))
        g4p = ctx.enter_context(tc.tile_pool(name="g4p", bufs=2, space="PSUM"))
        hp = ctx.enter_context(tc.tile_pool(name="hp", bufs=2))
        op = ctx.enter_context(tc.tile_pool(name="op", bufs=1, space="PSUM"))
        orp = ctx.enter_context(tc.tile_pool(name="orp", bufs=1))
        resall = orp.tile([8, N_PAIRS, 512], F32)

        # ---- all input DMAs issued upfront: X lives fully in SBUF (64 KB
        # per partition), so the sync queue never blocks on buffer reuse ----
        xtiles = []
        for c in range(2 * N_SLABS):
            xt = xp.tile([P, 4, HTILES * P], FP16, tag="x")
            base = c * CHUNK_ELEMS
            nc.sync.dma_start(
                xt,
                x_d[base:base + CHUNK_ELEMS].rearrange(
                    "(p k b) -> p k b", p=P, k=4),
            )
            xtiles.append(xt)

        # software-pipelined over slab PAIRS (2048 rows): front(pp) puts the
        # four half-slab thetaT blocks at partition offsets 0/32/64/96 of one
        # PSUM tile (col-tiled matmuls), one bulk ScalarE copy moves them to
        # SBUF, and back(pp) transposes 16 tiles per selector-matmul chunk.
        thsb_l = [None] * N_PAIRS
        psi_l = [None] * N_PAIRS

        # define junk rows of the rotating ttp PSUM buffers (never written by
        # the 8-row matmul outputs) so the selector's zeros multiply finite
        # values, not virgin-PSUM NaN patterns
        ttps2 = []
        for i in range(2):
            t = ttp.tile([P, 512], F32, tag=f"tt{i}")
            nc.vector.memset(t, 0.0)
            ttps2.append(t)

        def front(pp):
            ttps = ttps2[pp % 2]
            for q in range(4):
                xt = xtiles[pp * 4 + q]
                for k in range(4):
                    nc.tensor.matmul(
                        ttps[32 * q:32 * q + 8, :], wpk[:, k, :], xt[:, k, :],
                        start=(k == 0), stop=(k == 3),
                        tile_position=(0, 32 * q),
                    )
            thsb = tsb.tile([P, 512], FP16, tag="tt")
            nc.scalar.copy(thsb[0:104, :], ttps[0:104, :])
            thsb_l[pp] = thsb

        def back(pp):
            thsb = thsb_l[pp]
            # transpose back: ang[p, tau=c*4+q, j] = thetaT[32q+j, c*128+p]
            ang = angp.tile([P, 16, 2, 4], F32)
            av = ang.rearrange("p t d w -> p t (d w)")
            for c in range(4):
                nc.tensor.matmul(
                    av[:, c * 4:(c + 1) * 4, :].rearrange("p q j -> p (q j)"),
                    thsb[0:104, c * P:(c + 1) * P], sel[0:104, :],
                    start=True, stop=True,
                )

            # ---- bias + one-sided range-wrap + sin ----
            th2 = scr.tile([P, 16, 2, 4], F32, tag="th2")
            nc.vector.tensor_add(
                th2, ang,
                bia.unsqueeze(1).broadcast_to([P, 16, 2, 4]),
            )
            m1 = scr.tile([P, 16, 2, 4], F32, tag="m1")
            thw = scr.tile([P, 16, 2, 4], F32, tag="thw")
            nc.vector.tensor_scalar(
                m1, th2, PI, -2.0 * PI,
                op0=mybir.AluOpType.is_gt, op1=mybir.AluOpType.mult,
            )
            nc.vector.tensor_add(thw, th2, m1)
            sc = qp.tile([P, 16, 2, 4], FP16, tag="sc")
            nc.scalar.activation(sc, thw, mybir.ActivationFunctionType.Sin)

            # ---- psi' build: u = (c-s, c+s); psi = u0 x u1 x u2 x u3 ----
            ab = qp.tile([P, 16, 2, 4], FP16, tag="ab")
            nc.vector.tensor_sub(ab[:, :, 0, :], sc[:, :, 1, :], sc[:, :, 0, :])
            nc.vector.tensor_add(ab[:, :, 1, :], sc[:, :, 1, :], sc[:, :, 0, :])
            p01 = qp.tile([P, 16, 2, 2], FP16, tag="p01")
            nc.vector.tensor_mul(
                p01,
                ab[:, :, :, 0].unsqueeze(3).broadcast_to([P, 16, 2, 2]),
                ab[:, :, :, 1].unsqueeze(2).broadcast_to([P, 16, 2, 2]),
            )
            p23 = qp.tile([P, 16, 2, 2], FP16, tag="p23")
            nc.vector.tensor_mul(
                p23,
                ab[:, :, :, 2].unsqueeze(3).broadcast_to([P, 16, 2, 2]),
                ab[:, :, :, 3].unsqueeze(2).broadcast_to([P, 16, 2, 2]),
            )
            psi = qp.tile([P, 16, 4, 4], FP16, tag="psi")
            nc.vector.tensor_mul(
                psi,
                p01.rearrange("p t i j -> p t (i j)").unsqueeze(3)
                   .broadcast_to([P, 16, 4, 4]),
                p23.rearrange("p t k l -> p t (k l)").unsqueeze(2)
                   .broadcast_to([P, 16, 4, 4]),
            )
            psi_l[pp] = psi

        def back_b(pp):
            psi = psi_l[pp]
            # ---- transpose psi -> [tau*16+m, p] on the PE, bulk-copy ----
            pflat = psi.rearrange("p t a b -> p (t a b)")
            ptps = ptm.tile([P, 2, P], FP16)
            nc.tensor.transpose(ptps[:, 0, :], pflat[:, 0:128], ident)
            nc.tensor.transpose(ptps[:, 1, :], pflat[:, 128:256], ident)
            psiT = ptp.tile([P, 2, P], FP16)
            nc.vector.tensor_copy(psiT, ptps)

            # ---- g' = blockdiag(Q)^T psi'T ; h = g'^2 ; out = lam^T h ----
            g4 = g4p.tile([P, 4, P], F32)
            for u in range(2):
                nc.tensor.matmul(g4[:, 2 * u, :], gba, psiT[:, u, :],
                                 start=True, stop=True)
                nc.tensor.matmul(g4[:, 2 * u + 1, :], gbb, psiT[:, u, :],
                                 start=True, stop=True)
            h = hp.tile([P, 4, P], FP16)
            nc.scalar.square(h, g4)
            o = op.tile([8, 512], F32)
            nc.tensor.matmul(o, lam, h.rearrange("p a b -> p (a b)"),
                             start=True, stop=True)
            nc.vector.tensor_copy(resall[:, pp, :], o)
            nc.sync.dma_start(o_d[:, pp, :], resall[:, pp, :])

        for pp in range(N_PAIRS):
            front(pp)
            if 1 <= pp < N_PAIRS:
                back(pp - 1)
                if pp < N_PAIRS - 1:
                    back_b(pp - 1)
        # tail: start the last pair's A-chain before the prior pair's B-chain
        back(N_PAIRS - 1)
        back_b(N_PAIRS - 2)
        back_b(N_PAIRS - 1)

    nc.finalize()
    return nc


_NC_CACHE = {}


def _get_nc():
    if "nc" not in _NC_CACHE:
        _NC_CACHE["nc"] = build_bass()
    return _NC_CACHE["nc"]


def _host_consts(pre_w, pre_b, q_weights, post_w, post_b):
    W = np.asarray(pre_w, np.float64) / 2.0          # half-angle fold
    pb = np.asarray(pre_b, np.float64) / 2.0
    # wpk[p, k, j]: W2[j, 128k+p], W2 = [W; W] (dup for sin/cos columns)
    wpk = np.zeros((P, 4, 8), np.float16)
    for k in range(4):
        blk = W.T[P * k:P * (k + 1)]                 # [128, 4]
        wpk[:, k, 0:4] = blk.astype(np.float16)
        wpk[:, k, 4:8] = blk.astype(np.float16)
    b2 = np.stack([pb, pb + 0.5 * np.pi]).astype(np.float32)   # [2, 4]
    bias2 = np.broadcast_to(b2, (P, 2, 4)).copy()
    G, lamcol = _eigen_consts(
        np.asarray(q_weights, np.float64),
        np.asarray(post_w, np.float64),
        np.asarray(post_b, np.float64),
    )
    gba = np.zeros((P, 128), np.float16)
    gbb = np.zeros((P, 128), np.float16)
    for j in range(4):
        gba[j * 16:(j + 1) * 16, j * 32:(j + 1) * 32] = G.astype(np.float16)
        gbb[64 + j * 16:64 + (j + 1) * 16, j * 32:(j + 1) * 32] = \
            G.astype(np.float16)
    lam = np.zeros((P, 8), np.float16)
    for j in range(4):
        lam[j * 32:(j + 1) * 32, 2 * j:2 * j + 2] = lamcol.astype(np.float16)
    sel = np.zeros((P, 32), np.float16)
    for q in range(4):
        sel[32 * q:32 * q + 8, 8 * q:8 * q + 8] = np.eye(8, dtype=np.float16)
    cblob = np.concatenate(
        [wpk.reshape(P, 32), gba, gbb, lam,
         np.eye(P, dtype=np.float16), sel], axis=1).astype(np.float16)
    return {"cblob": np.ascontiguousarray(cblob), "bias2": bias2}


def _pack_x(x):
    """x [ROWS, F] f32 -> flat fp16, per-512-row chunk [P, 4, 512] with
    pack[p, k, b] = x16[c*512 + b, 128k + p]."""
    h = x.astype(np.float16)                          # [ROWS, 512]
    a = h.reshape(ROWS // 512, 512, 4, P)             # [c, b, k, p]
    return np.ascontiguousarray(a.transpose(0, 3, 2, 1)).reshape(-1)


def _unscramble(o):
    """o [8, N_PAIRS, 512] f32 -> [ROWS, 2].
    o[2j+k, pp, cc*128+p] -> row pp*2048 + j*512 + cc*128 + p."""
    o2 = o.reshape(4, 2, N_PAIRS, 4, 128)             # [j, k, pp, cc, p]
    return np.ascontiguousarray(
        o2.transpose(2, 0, 3, 4, 1)).reshape(ROWS, 2)  # pp, j, cc, p, k


def run(input_features, pre_w, pre_b, q_weights, post_w, post_b, **spmd_kwargs):
    x = np.asarray(input_features, dtype=np.float32)
    assert x.shape == (B_TOTAL, F_IN), x.shape
    consts = _host_consts(pre_w, pre_b, q_weights, post_w, post_b)
    in_maps = []
    for c in range(N_CORES):
        xt = _pack_x(x[c * ROWS:(c + 1) * ROWS])
        in_maps.append(dict(consts, xtp=xt))
    nc = _get_nc()
    r = run_bass_kernel_spmd(nc, in_maps, core_ids=list(range(N_CORES)),
                             **spmd_kwargs)
    out = np.concatenate(
        [_unscramble(np.asarray(r.results[c]["out"], np.float32))
         for c in range(N_CORES)],
        axis=0,
    )
    return out.astype(np.float32), r


def kernel(input_features, pre_w, pre_b, q_weights, post_w, post_b):
    out, _ = run(input_features, pre_w, pre_b, q_weights, post_w, post_b)
    return out


# revision 38
# speedup vs baseline: 1.0909x; 1.0909x over previous
"""DressedQuantumNet on 8 TRN2 NeuronCores (pure data parallel).

Math: pre-net angles th = X @ pre_w.T + pre_b.  After the H+RY(th) layer the
state is the real product state psi = kron_w u_w with
u_w = (cos(th_w/2) - sin(th_w/2), cos(th_w/2) + sin(th_w/2)) / sqrt(2),
and the rest of the circuit is a FIXED unitary V (depends only on q_weights).
Hence out_k = psi^T C_k psi + post_b_k with C_k = sum_w post_w[k,w]
Re(V^H Z_w V) real-symmetric.  Eigendecompose C_k = Q_k L_k Q_k^T on host:

  out_k = sum_r lam'_{k,r} * (Q_k^T psi')_r^2,   lam' = (lam + post_b_k)/16

using the unnormalized psi' (norm^2 = 16 exactly since each |u'_w|^2 = 2).

Device pipeline per 1024-row slab (batch rows on SBUF partitions):
  DMA fp16 X^T chunks -> PE matmul (X stationary, [W/2 | W/2] moving) ->
  angles in PSUM -> DVE bias-add + range-wrap -> ScalarE Sin LUT ->
  DVE psi' build (5 small fp16 ops) -> DMA-xbar transpose [128,128] ->
  PE matmul with block-diag [Q_0|Q_1] (4 tiles per matmul) -> ScalarE square
  -> PE matmul with block-diag lambda -> ScalarE copy -> DMA out.

This keeps the 81-term polynomial contraction OFF the (slow) vector engine:
the only DVE work is ~9 small elementwise ops per slab.  fp16 X halves HBM
traffic vs fp32 (theta error ~1e-3 << 2e-2 gate).
"""

from contextlib import ExitStack

import numpy as np

import concourse.bass as bass
import concourse.bacc as bacc_mod
import concourse.mybir as mybir
from concourse.bass_utils import run_bass_kernel_spmd
from concourse.tile import TileContext

N_CORES = 8
B_TOTAL = 65536
F_IN = 512
ROWS = B_TOTAL // N_CORES   # 8192 rows per core
P = 128
N_TILES = ROWS // P         # 64 row-tiles
SLAB = 8
N_SLABS = N_TILES // SLAB   # 8
N_PAIRS = N_SLABS // 2      # 4 slab-pairs (2048 rows each)
HTILES = 4                  # tiles per input DMA chunk (2 chunks per slab)

F32 = mybir.dt.float32
FP16 = mybir.dt.float16
PI = float(np.pi)

N_QUBITS, VAR_DEPTH = 4, 3


# ----------------------------------------------------------------- host math
def _gate_1q(g, w):
    ops = [np.eye(2, dtype=complex)] * N_QUBITS
    ops[w] = g
    U = ops[0]
    for i in range(1, N_QUBITS):
        U = np.kron(U, ops[i])
    return U


def _bit(i, w):  # wire 0 = most significant
    return (i >> (N_QUBITS - 1 - w)) & 1


def _cnot(c, t):
    M = np.zeros((16, 16), dtype=complex)
    for i in range(16):
        j = i ^ (1 << (N_QUBITS - 1 - t)) if _bit(i, c) else i
        M[j, i] = 1.0
    return M


def _ry(theta):
    c, s = np.cos(theta / 2), np.sin(theta / 2)
    return np.array([[c, -s], [s, c]], dtype=complex)


def _rz(theta):
    ph = np.exp(1j * theta / 2)
    return np.array([[np.conj(ph), 0], [0, ph]], dtype=complex)


def _fixed_unitary(qw):
    V = np.eye(16, dtype=complex)

    def app(Gm):
        nonlocal V
        V = Gm @ V

    def entangle():
        app(_cnot(0, 1)); app(_cnot(2, 3)); app(_cnot(1, 2))

    for k in range(VAR_DEPTH):
        entangle()
        for w in range(N_QUBITS):
            app(_gate_1q(_ry(qw[k, w]), w))
        for w in range(N_QUBITS):
            app(_gate_1q(_rz(qw[k, w]), w))
    for k in range(VAR_DEPTH):
        entangle()
        for w in range(N_QUBITS):
            app(_gate_1q(_ry(qw[k, w]), w))
        for w in range(N_QUBITS):
            app(_gate_1q(_rz(qw[3 + k, w]), w))
    entangle()
    return V


def _eigen_consts(q_weights, post_w, post_b):
    """G [16,32] = [Q_0 | Q_1]; lamcol [32,2]: (lam_k + post_b_k)/16."""
    V = _fixed_unitary(np.asarray(q_weights, dtype=np.float64))
    Gcols = []
    lamcol = np.zeros((32, 2), np.float64)
    for k in range(2):
        C = np.zeros((16, 16), dtype=complex)
        for w in range(N_QUBITS):
            z = np.array([1.0 - 2.0 * _bit(i, w) for i in range(16)])
            C += post_w[k, w] * (V.conj().T @ np.diag(z) @ V)
        M = C.real
        M = (M + M.T) / 2
        lam, Q = np.linalg.eigh(M)
        Gcols.append(Q)
        lamcol[16 * k:16 * (k + 1), k] = (lam + post_b[k]) / 16.0
    G = np.concatenate(Gcols, axis=1)  # [16, 32]
    return G, lamcol


# ------------------------------------------------------------- device kernel
def build_bass():
    nc = bacc_mod.Bacc(None, target_bir_lowering=False)
    # inputs: packed transposed fp16 X; per chunk c: [P, 4, 512] with
    # pack[p,k,b] = X16[c*512 + b, 128k + p]
    x_d = nc.dram_tensor("xtp", [ROWS * F_IN], FP16, kind="ExternalInput")
    # fp16 blob: [wpk(32)|gba(128)|gbb(128)|lam(8)|ident(128)|sel(32)] = [P,456]
    ch_d = nc.dram_tensor("cblob", [P, 456], FP16, kind="ExternalInput")
    b2_d = nc.dram_tensor("bias2", [P, 2, 4], F32, kind="ExternalInput")
    # out[2j+k, pp, (2u+v)*128+p] = out_row(pp*2048 + j*512 + (2u+v)*128 + p, k)
    o_d = nc.dram_tensor("out", [8, N_PAIRS, 512], F32, kind="ExternalOutput")

    CHUNK_ELEMS = P * 4 * HTILES * P  # 262144 elems per DMA chunk

    with TileContext(nc) as tc, ExitStack() as ctx:
        # constants arrive as one fp16 blob + one f32 blob (2 DMA issues)
        const = ctx.enter_context(tc.tile_pool(name="const", bufs=1))
        ch = const.tile([P, 456], FP16)
        nc.scalar.dma_start(ch, ch_d[:])
        wpk = ch[:, 0:32].rearrange("p (k j) -> p k j", k=4)
        gba = ch[:, 32:160]
        gbb = ch[:, 160:288]
        lam = ch[:, 288:296]
        ident = ch[:, 296:424]
        sel = ch[:, 424:456]
        bia = const.tile([P, 2, 4], F32)
        nc.scalar.dma_start(bia, b2_d[:])

        xp = ctx.enter_context(tc.tile_pool(name="xin", bufs=16))
        ttp = ctx.enter_context(tc.tile_pool(name="ttp", bufs=1, space="PSUM"))
        tsb = ctx.enter_context(tc.tile_pool(name="tsb", bufs=3))
        angp = ctx.enter_context(tc.tile_pool(name="angp", bufs=1, space="PSUM"))
        scr = ctx.enter_context(tc.tile_pool(name="scr", bufs=2))
        qp = ctx.enter_context(tc.tile_pool(name="qp", bufs=3))
        ptm = ctx.enter_context(tc.tile_pool(name="ptm", bufs=2, space="PSUM"))
        ptp = ctx.enter_context(tc.tile_pool(name="ptp", bufs=2))
        g4p = ctx.enter_context(tc.tile_pool(name="g4p", bufs=2, space="PSUM"))
        hp = ctx.enter_context(tc.tile_pool(name="hp", bufs=2))
        op = ctx.enter_context(tc.tile_pool(name="op", bufs=1, space="PSUM"))
        orp = ctx.enter_context(tc.tile_pool(name="orp", bufs=1))
        resall = orp.tile([8, N_PAIRS, 512], F32)

        # ---- all input DMAs issued upfront: X lives fully in SBUF (64 KB
        # per partition), so the sync queue never blocks on buffer reuse ----
        xtiles = []
        for c in range(2 * N_SLABS):
            xt = xp.tile([P, 4, HTILES * P], FP16, tag="x")
            base = c * CHUNK_ELEMS
            nc.sync.dma_start(
                xt,
                x_d[base:base + CHUNK_ELEMS].rearrange(
                    "(p k b) -> p k b", p=P, k=4),
            )
            xtiles.append(xt)

        # software-pipelined over slab PAIRS (2048 rows): front(pp) puts the
        # four half-slab thetaT blocks at partition offsets 0/32/64/96 of one
        # PSUM tile (col-tiled matmuls), one bulk ScalarE copy moves them to
        # SBUF, and back(pp) transposes 16 tiles per selector-matmul chunk.
        thsb_l = [None] * N_PAIRS
        psi_l = [None] * N_PAIRS

        # define junk rows of the rotating ttp PSUM buffers (never written by
        # the 8-row matmul outputs) so the selector's zeros multiply finite
        # values, not virgin-PSUM NaN patterns
        ttps2 = []
        for i in range(2):
            t = ttp.tile([P, 512], F32, tag=f"tt{i}")
            nc.vector.memset(t, 0.0)
            ttps2.append(t)

        def front(pp):
            ttps = ttps2[pp % 2]
            for q in range(4):
                xt = xtiles[pp * 4 + q]
                for k in range(4):
                    nc.tensor.matmul(
                        ttps[32 * q:32 * q + 8, :], wpk[:, k, :], xt[:, k, :],
                        start=(k == 0), stop=(k == 3),
                        tile_position=(0, 32 * q),
                    )
            thsb = tsb.tile([P, 512], FP16, tag="tt")
            nc.scalar.copy(thsb[0:104, :], ttps[0:104, :])
            thsb_l[pp] = thsb

        def back(pp):
            thsb = thsb_l[pp]
            # transpose back: ang[p, tau=c*4+q, j] = thetaT[32q+j, c*128+p]
            ang = angp.tile([P, 16, 2, 4], F32)
            av = ang.rearrange("p t d w -> p t (d w)")
            for c in range(4):
                nc.tensor.matmul(
                    av[:, c * 4:(c + 1) * 4, :].rearrange("p q j -> p (q j)"),
                    thsb[0:104, c * P:(c + 1) * P], sel[0:104, :],
                    start=True, stop=True,
                )

            # ---- bias + one-sided range-wrap + sin ----
            th2 = scr.tile([P, 16, 2, 4], F32, tag="th2")
            nc.vector.tensor_add(
                th2, ang,
                bia.unsqueeze(1).broadcast_to([P, 16, 2, 4]),
            )
            m1 = scr.tile([P, 16, 2, 4], F32, tag="m1")
            thw = scr.tile([P, 16, 2, 4], F32, tag="thw")
            nc.vector.tensor_scalar(
                m1, th2, PI, -2.0 * PI,
                op0=mybir.AluOpType.is_gt, op1=mybir.AluOpType.mult,
            )
            nc.vector.tensor_add(thw, th2, m1)
            sc = qp.tile([P, 16, 2, 4], FP16, tag="sc")
            nc.scalar.activation(sc, thw, mybir.ActivationFunctionType.Sin)

            # ---- psi' build: u = (c-s, c+s); psi = u0 x u1 x u2 x u3 ----
            ab = qp.tile([P, 16, 2, 4], FP16, tag="ab")
            nc.vector.tensor_sub(ab[:, :, 0, :], sc[:, :, 1, :], sc[:, :, 0, :])
            nc.vector.tensor_add(ab[:, :, 1, :], sc[:, :, 1, :], sc[:, :, 0, :])
            p01 = qp.tile([P, 16, 2, 2], FP16, tag="p01")
            nc.vector.tensor_mul(
                p01,
                ab[:, :, :, 0].unsqueeze(3).broadcast_to([P, 16, 2, 2]),
                ab[:, :, :, 1].unsqueeze(2).broadcast_to([P, 16, 2, 2]),
            )
            p23 = qp.tile([P, 16, 2, 2], FP16, tag="p23")
            nc.vector.tensor_mul(
                p23,
                ab[:, :, :, 2].unsqueeze(3).broadcast_to([P, 16, 2, 2]),
                ab[:, :, :, 3].unsqueeze(2).broadcast_to([P, 16, 2, 2]),
            )
            psi = qp.tile([P, 16, 4, 4], FP16, tag="psi")
            nc.vector.tensor_mul(
                psi,
                p01.rearrange("p t i j -> p t (i j)").unsqueeze(3)
                   .broadcast_to([P, 16, 4, 4]),
                p23.rearrange("p t k l -> p t (k l)").unsqueeze(2)
                   .broadcast_to([P, 16, 4, 4]),
            )
            psi_l[pp] = psi

        def back_b(pp):
            psi = psi_l[pp]
            # ---- transpose psi -> [tau*16+m, p] on the PE, bulk-copy ----
            pflat = psi.rearrange("p t a b -> p (t a b)")
            ptps = ptm.tile([P, 2, P], FP16)
            nc.tensor.transpose(ptps[:, 0, :], pflat[:, 0:128], ident)
            nc.tensor.transpose(ptps[:, 1, :], pflat[:, 128:256], ident)
            psiT = ptp.tile([P, 2, P], FP16)
            nc.vector.tensor_copy(psiT, ptps)

            # ---- g' = blockdiag(Q)^T psi'T ; h = g'^2 ; out = lam^T h ----
            g4 = g4p.tile([P, 4, P], F32)
            for u in range(2):
                nc.tensor.matmul(g4[:, 2 * u, :], gba, psiT[:, u, :],
                                 start=True, stop=True)
                nc.tensor.matmul(g4[:, 2 * u + 1, :], gbb, psiT[:, u, :],
                                 start=True, stop=True)
            h = hp.tile([P, 4, P], FP16)
            nc.scalar.square(h, g4)
            o = op.tile([8, 512], F32)
            nc.tensor.matmul(o, lam, h.rearrange("p a b -> p (a b)"),
                             start=True, stop=True)
            nc.vector.tensor_copy(resall[:, pp, :], o)
            nc.sync.dma_start(o_d[:, pp, :], resall[:, pp, :])

        for pp in range(N_PAIRS + 1):
            if pp < N_PAIRS:
                front(pp)
            if pp >= 1:
                back(pp - 1)
                back_b(pp - 1)

    nc.finalize()
    return nc


_NC_CACHE = {}


def _get_nc():
    if "nc" not in _NC_CACHE:
        _NC_CACHE["nc"] = build_bass()
    return _NC_CACHE["nc"]


def _host_consts(pre_w, pre_b, q_weights, post_w, post_b):
    W = np.asarray(pre_w, np.float64) / 2.0          # half-angle fold
    pb = np.asarray(pre_b, np.float64) / 2.0
    # wpk[p, k, j]: W2[j, 128k+p], W2 = [W; W] (dup for sin/cos columns)
    wpk = np.zeros((P, 4, 8), np.float16)
    for k in range(4):
        blk = W.T[P * k:P * (k + 1)]                 # [128, 4]
        wpk[:, k, 0:4] = blk.astype(np.float16)
        wpk[:, k, 4:8] = blk.astype(np.float16)
    b2 = np.stack([pb, pb + 0.5 * np.pi]).astype(np.float32)   # [2, 4]
    bias2 = np.broadcast_to(b2, (P, 2, 4)).copy()
    G, lamcol = _eigen_consts(
        np.asarray(q_weights, np.float64),
        np.asarray(post_w, np.float64),
        np.asarray(post_b, np.float64),
    )
    gba = np.zeros((P, 128), np.float16)
    gbb = np.zeros((P, 128), np.float16)
    for j in range(4):
        gba[j * 16:(j + 1) * 16, j * 32:(j + 1) * 32] = G.astype(np.float16)
        gbb[64 + j * 16:64 + (j + 1) * 16, j * 32:(j + 1) * 32] = \
            G.astype(np.float16)
    lam = np.zeros((P, 8), np.float16)
    for j in range(4):
        lam[j * 32:(j + 1) * 32, 2 * j:2 * j + 2] = lamcol.astype(np.float16)
    sel = np.zeros((P, 32), np.float16)
    for q in range(4):
        sel[32 * q:32 * q + 8, 8 * q:8 * q + 8] = np.eye(8, dtype=np.float16)
    cblob = np.concatenate(
        [wpk.reshape(P, 32), gba, gbb, lam,
         np.eye(P, dtype=np.float16), sel], axis=1).astype(np.float16)
    return {"cblob": np.ascontiguousarray(cblob), "bias2": bias2}


def _pack_x(x):
    """x [ROWS, F] f32 -> flat fp16, per-512-row chunk [P, 4, 512] with
    pack[p, k, b] = x16[c*512 + b, 128k + p]."""
    h = x.astype(np.float16)                          # [ROWS, 512]
    a = h.reshape(ROWS // 512, 512, 4, P)             # [c, b, k, p]
    return np.ascontiguousarray(a.transpose(0, 3, 2, 1)).reshape(-1)


def _unscramble(o):
    """o [8, N_PAIRS, 512] f32 -> [ROWS, 2].
    o[2j+k, pp, cc*128+p] -> row pp*2048 + j*512 + cc*128 + p."""
    o2 = o.reshape(4, 2, N_PAIRS, 4, 128)             # [j, k, pp, cc, p]
    return np.ascontiguousarray(
        o2.transpose(2, 0, 3, 4, 1)).reshape(ROWS, 2)  # pp, j, cc, p, k


def run(input_features, pre_w, pre_b, q_weights, post_w, post_b, **spmd_kwargs):
    x = np.asarray(input_features, dtype=np.float32)
    assert x.shape == (B_TOTAL, F_IN), x.shape
    consts = _host_consts(pre_w, pre_b, q_weights, post_w, post_b)
    in_maps = []
    for c in range(N_CORES):
        xt = _pack_x(x[c * ROWS:(c + 1) * ROWS])
        in_maps.append(dict(consts, xtp=xt))
    nc = _get_nc()
    r = run_bass_kernel_spmd(nc, in_maps, core_ids=list(range(N_CORES)),
                             **spmd_kwargs)
    out = np.concatenate(
        [_unscramble(np.asarray(r.results[c]["out"], np.float32))
         for c in range(N_CORES)],
        axis=0,
    )
    return out.astype(np.float32), r


def kernel(input_features, pre_w, pre_b, q_weights, post_w, post_b):
    out, _ = run(input_features, pre_w, pre_b, q_weights, post_w, post_b)
    return out


# revision 39
# speedup vs baseline: 1.0967x; 1.0054x over previous
"""DressedQuantumNet on 8 TRN2 NeuronCores (pure data parallel).

Math: pre-net angles th = X @ pre_w.T + pre_b.  After the H+RY(th) layer the
state is the real product state psi = kron_w u_w with
u_w = (cos(th_w/2) - sin(th_w/2), cos(th_w/2) + sin(th_w/2)) / sqrt(2),
and the rest of the circuit is a FIXED unitary V (depends only on q_weights).
Hence out_k = psi^T C_k psi + post_b_k with C_k = sum_w post_w[k,w]
Re(V^H Z_w V) real-symmetric.  Eigendecompose C_k = Q_k L_k Q_k^T on host:

  out_k = sum_r lam'_{k,r} * (Q_k^T psi')_r^2,   lam' = (lam + post_b_k)/16

using the unnormalized psi' (norm^2 = 16 exactly since each |u'_w|^2 = 2).

Device pipeline per 1024-row slab (batch rows on SBUF partitions):
  DMA fp16 X^T chunks -> PE matmul (X stationary, [W/2 | W/2] moving) ->
  angles in PSUM -> DVE bias-add + range-wrap -> ScalarE Sin LUT ->
  DVE psi' build (5 small fp16 ops) -> DMA-xbar transpose [128,128] ->
  PE matmul with block-diag [Q_0|Q_1] (4 tiles per matmul) -> ScalarE square
  -> PE matmul with block-diag lambda -> ScalarE copy -> DMA out.

This keeps the 81-term polynomial contraction OFF the (slow) vector engine:
the only DVE work is ~9 small elementwise ops per slab.  fp16 X halves HBM
traffic vs fp32 (theta error ~1e-3 << 2e-2 gate).
"""

from contextlib import ExitStack

import numpy as np

import concourse.bass as bass
import concourse.bacc as bacc_mod
import concourse.mybir as mybir
from concourse.bass_utils import run_bass_kernel_spmd
from concourse.tile import TileContext

N_CORES = 8
B_TOTAL = 65536
F_IN = 512
ROWS = B_TOTAL // N_CORES   # 8192 rows per core
P = 128
N_TILES = ROWS // P         # 64 row-tiles
SLAB = 8
N_SLABS = N_TILES // SLAB   # 8
N_PAIRS = N_SLABS // 2      # 4 slab-pairs (2048 rows each)
HTILES = 4                  # tiles per input DMA chunk (2 chunks per slab)

F32 = mybir.dt.float32
FP16 = mybir.dt.float16
PI = float(np.pi)

N_QUBITS, VAR_DEPTH = 4, 3


# ----------------------------------------------------------------- host math
def _gate_1q(g, w):
    ops = [np.eye(2, dtype=complex)] * N_QUBITS
    ops[w] = g
    U = ops[0]
    for i in range(1, N_QUBITS):
        U = np.kron(U, ops[i])
    return U


def _bit(i, w):  # wire 0 = most significant
    return (i >> (N_QUBITS - 1 - w)) & 1


def _cnot(c, t):
    M = np.zeros((16, 16), dtype=complex)
    for i in range(16):
        j = i ^ (1 << (N_QUBITS - 1 - t)) if _bit(i, c) else i
        M[j, i] = 1.0
    return M


def _ry(theta):
    c, s = np.cos(theta / 2), np.sin(theta / 2)
    return np.array([[c, -s], [s, c]], dtype=complex)


def _rz(theta):
    ph = np.exp(1j * theta / 2)
    return np.array([[np.conj(ph), 0], [0, ph]], dtype=complex)


def _fixed_unitary(qw):
    V = np.eye(16, dtype=complex)

    def app(Gm):
        nonlocal V
        V = Gm @ V

    def entangle():
        app(_cnot(0, 1)); app(_cnot(2, 3)); app(_cnot(1, 2))

    for k in range(VAR_DEPTH):
        entangle()
        for w in range(N_QUBITS):
            app(_gate_1q(_ry(qw[k, w]), w))
        for w in range(N_QUBITS):
            app(_gate_1q(_rz(qw[k, w]), w))
    for k in range(VAR_DEPTH):
        entangle()
        for w in range(N_QUBITS):
            app(_gate_1q(_ry(qw[k, w]), w))
        for w in range(N_QUBITS):
            app(_gate_1q(_rz(qw[3 + k, w]), w))
    entangle()
    return V


def _eigen_consts(q_weights, post_w, post_b):
    """G [16,32] = [Q_0 | Q_1]; lamcol [32,2]: (lam_k + post_b_k)/16."""
    V = _fixed_unitary(np.asarray(q_weights, dtype=np.float64))
    Gcols = []
    lamcol = np.zeros((32, 2), np.float64)
    for k in range(2):
        C = np.zeros((16, 16), dtype=complex)
        for w in range(N_QUBITS):
            z = np.array([1.0 - 2.0 * _bit(i, w) for i in range(16)])
            C += post_w[k, w] * (V.conj().T @ np.diag(z) @ V)
        M = C.real
        M = (M + M.T) / 2
        lam, Q = np.linalg.eigh(M)
        Gcols.append(Q)
        lamcol[16 * k:16 * (k + 1), k] = (lam + post_b[k]) / 16.0
    G = np.concatenate(Gcols, axis=1)  # [16, 32]
    return G, lamcol


# ------------------------------------------------------------- device kernel
def build_bass():
    nc = bacc_mod.Bacc(None, target_bir_lowering=False)
    # inputs: packed transposed fp16 X; per chunk c: [P, 4, 512] with
    # pack[p,k,b] = X16[c*512 + b, 128k + p]
    x_d = nc.dram_tensor("xtp", [ROWS * F_IN], FP16, kind="ExternalInput")
    # fp16 blob: [wpk(32)|gba(128)|gbb(128)|lam(8)|ident(128)|sel(32)] = [P,456]
    ch_d = nc.dram_tensor("cblob", [P, 456], FP16, kind="ExternalInput")
    b2_d = nc.dram_tensor("bias2", [P, 2, 4], F32, kind="ExternalInput")
    # out[2j+k, pp, (2u+v)*128+p] = out_row(pp*2048 + j*512 + (2u+v)*128 + p, k)
    o_d = nc.dram_tensor("out", [8, N_PAIRS, 512], F32, kind="ExternalOutput")

    CHUNK_ELEMS = P * 4 * HTILES * P  # 262144 elems per DMA chunk

    with TileContext(nc) as tc, ExitStack() as ctx:
        # constants arrive as one fp16 blob + one f32 blob (2 DMA issues)
        const = ctx.enter_context(tc.tile_pool(name="const", bufs=1))
        ch = const.tile([P, 456], FP16)
        nc.scalar.dma_start(ch, ch_d[:])
        wpk = ch[:, 0:32].rearrange("p (k j) -> p k j", k=4)
        gba = ch[:, 32:160]
        gbb = ch[:, 160:288]
        lam = ch[:, 288:296]
        ident = ch[:, 296:424]
        sel = ch[:, 424:456]
        bia = const.tile([P, 2, 4], F32)
        nc.scalar.dma_start(bia, b2_d[:])

        xp = ctx.enter_context(tc.tile_pool(name="xin", bufs=16))
        ttp = ctx.enter_context(tc.tile_pool(name="ttp", bufs=1, space="PSUM"))
        tsb = ctx.enter_context(tc.tile_pool(name="tsb", bufs=3))
        angp = ctx.enter_context(tc.tile_pool(name="angp", bufs=1, space="PSUM"))
        scr = ctx.enter_context(tc.tile_pool(name="scr", bufs=2))
        qp = ctx.enter_context(tc.tile_pool(name="qp", bufs=3))
        ptm = ctx.enter_context(tc.tile_pool(name="ptm", bufs=2, space="PSUM"))
        ptp = ctx.enter_context(tc.tile_pool(name="ptp", bufs=2))
        g4p = ctx.enter_context(tc.tile_pool(name="g4p", bufs=2, space="PSUM"))
        hp = ctx.enter_context(tc.tile_pool(name="hp", bufs=2))
        op = ctx.enter_context(tc.tile_pool(name="op", bufs=1, space="PSUM"))
        orp = ctx.enter_context(tc.tile_pool(name="orp", bufs=1))
        resall = orp.tile([8, N_PAIRS, 512], F32)

        # ---- all input DMAs issued upfront: X lives fully in SBUF (64 KB
        # per partition), so the sync queue never blocks on buffer reuse ----
        xtiles = []
        for c in range(2 * N_SLABS):
            xt = xp.tile([P, 4, HTILES * P], FP16, tag="x")
            base = c * CHUNK_ELEMS
            nc.sync.dma_start(
                xt,
                x_d[base:base + CHUNK_ELEMS].rearrange(
                    "(p k b) -> p k b", p=P, k=4),
            )
            xtiles.append(xt)

        # software-pipelined over slab PAIRS (2048 rows): front(pp) puts the
        # four half-slab thetaT blocks at partition offsets 0/32/64/96 of one
        # PSUM tile (col-tiled matmuls), one bulk ScalarE copy moves them to
        # SBUF, and back(pp) transposes 16 tiles per selector-matmul chunk.
        thsb_l = [None] * N_PAIRS
        psi_l = [None] * N_PAIRS

        # define junk rows of the rotating ttp PSUM buffers (never written by
        # the 8-row matmul outputs) so the selector's zeros multiply finite
        # values, not virgin-PSUM NaN patterns
        ttps2 = []
        for i in range(2):
            t = ttp.tile([P, 512], F32, tag=f"tt{i}")
            nc.vector.memset(t, 0.0)
            ttps2.append(t)

        def front(pp):
            ttps = ttps2[pp % 2]
            for q in range(4):
                xt = xtiles[pp * 4 + q]
                for k in range(4):
                    nc.tensor.matmul(
                        ttps[32 * q:32 * q + 8, :], wpk[:, k, :], xt[:, k, :],
                        start=(k == 0), stop=(k == 3),
                        tile_position=(0, 32 * q),
                    )
            thsb = tsb.tile([P, 512], FP16, tag="tt")
            nc.scalar.copy(thsb[0:104, :], ttps[0:104, :])
            thsb_l[pp] = thsb

        def back(pp):
            thsb = thsb_l[pp]
            # transpose back: ang[p, tau=c*4+q, j] = thetaT[32q+j, c*128+p]
            ang = angp.tile([P, 16, 2, 4], F32)
            av = ang.rearrange("p t d w -> p t (d w)")
            for c in range(4):
                nc.tensor.matmul(
                    av[:, c * 4:(c + 1) * 4, :].rearrange("p q j -> p (q j)"),
                    thsb[0:104, c * P:(c + 1) * P], sel[0:104, :],
                    start=True, stop=True,
                )

            # ---- bias + one-sided range-wrap + sin ----
            th2 = scr.tile([P, 16, 2, 4], F32, tag="th2")
            nc.vector.tensor_add(
                th2, ang,
                bia.unsqueeze(1).broadcast_to([P, 16, 2, 4]),
            )
            m1 = scr.tile([P, 16, 2, 4], F32, tag="m1")
            thw = scr.tile([P, 16, 2, 4], F32, tag="thw")
            nc.vector.tensor_scalar(
                m1, th2, PI, -2.0 * PI,
                op0=mybir.AluOpType.is_gt, op1=mybir.AluOpType.mult,
            )
            nc.vector.tensor_add(thw, th2, m1)
            sc = qp.tile([P, 16, 2, 4], FP16, tag="sc")
            nc.scalar.activation(sc, thw, mybir.ActivationFunctionType.Sin)

            # ---- psi' build: u = (c-s, c+s); psi = u0 x u1 x u2 x u3 ----
            ab = qp.tile([P, 16, 2, 4], FP16, tag="ab")
            nc.vector.tensor_sub(ab[:, :, 0, :], sc[:, :, 1, :], sc[:, :, 0, :])
            nc.vector.tensor_add(ab[:, :, 1, :], sc[:, :, 1, :], sc[:, :, 0, :])
            p01 = qp.tile([P, 16, 2, 2], FP16, tag="p01")
            nc.vector.tensor_mul(
                p01,
                ab[:, :, :, 0].unsqueeze(3).broadcast_to([P, 16, 2, 2]),
                ab[:, :, :, 1].unsqueeze(2).broadcast_to([P, 16, 2, 2]),
            )
            p23 = qp.tile([P, 16, 2, 2], FP16, tag="p23")
            nc.vector.tensor_mul(
                p23,
                ab[:, :, :, 2].unsqueeze(3).broadcast_to([P, 16, 2, 2]),
                ab[:, :, :, 3].unsqueeze(2).broadcast_to([P, 16, 2, 2]),
            )
            psi = qp.tile([P, 16, 4, 4], FP16, tag="psi")
            nc.vector.tensor_mul(
                psi,
                p01.rearrange("p t i j -> p t (i j)").unsqueeze(3)
                   .broadcast_to([P, 16, 4, 4]),
                p23.rearrange("p t k l -> p t (k l)").unsqueeze(2)
                   .broadcast_to([P, 16, 4, 4]),
            )
            psi_l[pp] = psi

        def back_b(pp):
            psi = psi_l[pp]
            # ---- transpose psi -> [tau*16+m, p] on the PE, bulk-copy ----
            pflat = psi.rearrange("p t a b -> p (t a b)")
            ptps = ptm.tile([P, 2, P], FP16)
            nc.tensor.transpose(ptps[:, 0, :], pflat[:, 0:128], ident)
            nc.tensor.transpose(ptps[:, 1, :], pflat[:, 128:256], ident)
            psiT = ptp.tile([P, 2, P], FP16)
            nc.vector.tensor_copy(psiT, ptps)

            # ---- g' = blockdiag(Q)^T psi'T ; h = g'^2 ; out = lam^T h ----
            g4 = g4p.tile([P, 4, P], F32)
            for u in range(2):
                nc.tensor.matmul(g4[:, 2 * u, :], gba, psiT[:, u, :],
                                 start=True, stop=True)
                nc.tensor.matmul(g4[:, 2 * u + 1, :], gbb, psiT[:, u, :],
                                 start=True, stop=True)
            h = hp.tile([P, 4, P], FP16)
            nc.scalar.square(h, g4)
            o = op.tile([8, 512], F32)
            nc.tensor.matmul(o, lam, h.rearrange("p a b -> p (a b)"),
                             start=True, stop=True)
            nc.scalar.copy(resall[:, pp, :], o)
            nc.sync.dma_start(o_d[:, pp, :], resall[:, pp, :])

        for pp in range(N_PAIRS + 1):
            if pp < N_PAIRS:
                front(pp)
            if pp >= 1:
                back(pp - 1)
                back_b(pp - 1)

    nc.finalize()
    return nc


_NC_CACHE = {}


def _get_nc():
    if "nc" not in _NC_CACHE:
        _NC_CACHE["nc"] = build_bass()
    return _NC_CACHE["nc"]


def _host_consts(pre_w, pre_b, q_weights, post_w, post_b):
    W = np.asarray(pre_w, np.float64) / 2.0          # half-angle fold
    pb = np.asarray(pre_b, np.float64) / 2.0
    # wpk[p, k, j]: W2[j, 128k+p], W2 = [W; W] (dup for sin/cos columns)
    wpk = np.zeros((P, 4, 8), np.float16)
    for k in range(4):
        blk = W.T[P * k:P * (k + 1)]                 # [128, 4]
        wpk[:, k, 0:4] = blk.astype(np.float16)
        wpk[:, k, 4:8] = blk.astype(np.float16)
    b2 = np.stack([pb, pb + 0.5 * np.pi]).astype(np.float32)   # [2, 4]
    bias2 = np.broadcast_to(b2, (P, 2, 4)).copy()
    G, lamcol = _eigen_consts(
        np.asarray(q_weights, np.float64),
        np.asarray(post_w, np.float64),
        np.asarray(post_b, np.float64),
    )
    gba = np.zeros((P, 128), np.float16)
    gbb = np.zeros((P, 128), np.float16)
    for j in range(4):
        gba[j * 16:(j + 1) * 16, j * 32:(j + 1) * 32] = G.astype(np.float16)
        gbb[64 + j * 16:64 + (j + 1) * 16, j * 32:(j + 1) * 32] = \
            G.astype(np.float16)
    lam = np.zeros((P, 8), np.float16)
    for j in range(4):
        lam[j * 32:(j + 1) * 32, 2 * j:2 * j + 2] = lamcol.astype(np.float16)
    sel = np.zeros((P, 32), np.float16)
    for q in range(4):
        sel[32 * q:32 * q + 8, 8 * q:8 * q + 8] = np.eye(8, dtype=np.float16)
    cblob = np.concatenate(
        [wpk.reshape(P, 32), gba, gbb, lam,
         np.eye(P, dtype=np.float16), sel], axis=1).astype(np.float16)
    return {"cblob": np.ascontiguousarray(cblob), "bias2": bias2}


def _pack_x(x):
    """x [ROWS, F] f32 -> flat fp16, per-512-row chunk [P, 4, 512] with
    pack[p, k, b] = x16[c*512 + b, 128k + p]."""
    h = x.astype(np.float16)                          # [ROWS, 512]
    a = h.reshape(ROWS // 512, 512, 4, P)             # [c, b, k, p]
    return np.ascontiguousarray(a.transpose(0, 3, 2, 1)).reshape(-1)


def _unscramble(o):
    """o [8, N_PAIRS, 512] f32 -> [ROWS, 2].
    o[2j+k, pp, cc*128+p] -> row pp*2048 + j*512 + cc*128 + p."""
    o2 = o.reshape(4, 2, N_PAIRS, 4, 128)             # [j, k, pp, cc, p]
    return np.ascontiguousarray(
        o2.transpose(2, 0, 3, 4, 1)).reshape(ROWS, 2)  # pp, j, cc, p, k


def run(input_features, pre_w, pre_b, q_weights, post_w, post_b, **spmd_kwargs):
    x = np.asarray(input_features, dtype=np.float32)
    assert x.shape == (B_TOTAL, F_IN), x.shape
    consts = _host_consts(pre_w, pre_b, q_weights, post_w, post_b)
    in_maps = []
    for c in range(N_CORES):
        xt = _pack_x(x[c * ROWS:(c + 1) * ROWS])
        in_maps.append(dict(consts, xtp=xt))
    nc = _get_nc()
    r = run_bass_kernel_spmd(nc, in_maps, core_ids=list(range(N_CORES)),
                             **spmd_kwargs)
    out = np.concatenate(
        [_unscramble(np.asarray(r.results[c]["out"], np.float32))
         for c in range(N_CORES)],
        axis=0,
    )
    return out.astype(np.float32), r


def kernel(input_features, pre_w, pre_b, q_weights, post_w, post_b):
    out, _ = run(input_features, pre_w, pre_b, q_weights, post_w, post_b)
    return out
